# revision 1
# baseline (speedup 1.0000x reference)
"""CrossTransformer Trainium2 kernel — 8 NeuronCores.

Sharding: core c = (batch b = c//2, head-pair hg = c%2).  Attention is
head-parallel (2 heads/core, fp32r matmuls, exp on ACT with fused
row-sum accum); out-proj + FFN are token-parallel (half of the 2048
tokens per core) after an intra-pair AllToAll of the attention output.

Both softmax directions run the same "column-softmax" pipeline with
swapped inputs (m0 = m1-path(x1, x0)); each path's softmax denominator
is the opposite path's exp row-sum (E_ba = E_ab^T).
"""
import numpy as np

B, NT, E, H, D = 4, 2048, 256, 4, 64
HPC = 2            # heads per core
TH = NT // 2       # token half
HID = 2 * E        # FFN hidden (512)
KCH = E // 128     # 128-chunks of E (2)
N_CORES = 8
LN_EPS = 1e-5

_cache = {}


def _build():
    import concourse.bass as bass
    import concourse.tile as tile
    from concourse import bacc
    import concourse.mybir as mybir

    dt = mybir.dt
    AF = mybir.ActivationFunctionType
    OP = mybir.AluOpType
    f32, f32r = dt.float32, dt.float32r

    nc = bacc.Bacc("TRN2", target_bir_lowering=False, debug=False,
                   num_devices=N_CORES)

    def din(name, shape):
        return nc.dram_tensor(name, shape, f32, kind="ExternalInput").ap()

    x0t = din("x0t", [E, NT])          # x0[b].T
    x1t = din("x1t", [E, NT])
    xf_r = [din(f"x{d}t_ffn", [E, TH]) for d in (0, 1)]   # my token half
    wqk = din("wqk", [E, 128])         # pre-scaled, this core's heads
    bqk = din("bqk", [128, 1])
    wv = din("wv", [E, 256])
    bv = din("bv", [128, 1])
    wo = din("wo", [E, E])
    bo = din("bo", [E, 1])
    w1 = din("w1", [HID, HID])
    w1bar = din("w1bar", [HID, 1])
    b1 = din("b1", [HID, 1])
    b1bar = din("b1bar", [1, 1])
    lng = din("lng", [HID, 1])
    lnb = din("lnb", [HID, 1])
    w2 = din("w2", [HID, E])
    b2 = din("b2", [E, 1])
    ident = din("ident", [128, 128])   # identity matrix
    ones = din("ones", [128, 1])

    outs = [nc.dram_tensor(f"out{d}t", [E, TH], f32, kind="ExternalOutput").ap()
            for d in (0, 1)]

    with tile.TileContext(nc) as tc:
        _body(nc, tc, bass, mybir, tile,
              dict(x0t=x0t, x1t=x1t, xf_r=xf_r, wqk=wqk, bqk=bqk, wv=wv,
                   bv=bv, wo=wo, bo=bo, w1=w1, w1bar=w1bar, b1=b1,
                   b1bar=b1bar, lng=lng, lnb=lnb, w2=w2, b2=b2,
                   ident=ident, ones=ones, outs=outs))
    nc.compile()
    return nc


def _body(nc, tc, bass, mybir, tile, t):
    from contextlib import ExitStack
    dt = mybir.dt
    AF = mybir.ActivationFunctionType
    OP = mybir.AluOpType
    f32, f32r = dt.float32, dt.float32r

    es = ExitStack()
    with es:
        wpool = es.enter_context(tc.tile_pool(name="weights", bufs=1))
        dram = es.enter_context(tc.tile_pool(name="dram", bufs=1, space="DRAM"))

        # ---- weight loads (SWDGE casts fp32 -> fp32r where PE consumes) ----
        def load_r(ap_src, p, fshape, tag):
            til = wpool.tile([p, fshape], f32r, tag=tag, name=tag)
            nc.gpsimd.dma_start(til[:], ap_src)
            return til

        def load_f(ap_src, p, fshape, tag):
            til = wpool.tile([p, fshape], f32, tag=tag, name=tag)
            nc.sync.dma_start(til[:], ap_src)
            return til

        wqk_t = [load_r(t["wqk"][k * 128:(k + 1) * 128, :], 128, 128, f"wqk{k}") for k in range(KCH)]
        wv_t = [load_r(t["wv"][k * 128:(k + 1) * 128, :], 128, 256, f"wv{k}") for k in range(KCH)]
        wo_t = [[load_r(t["wo"][k * 128:(k + 1) * 128, m * 128:(m + 1) * 128], 128, 128, f"wo{k}{m}")
                 for m in range(2)] for k in range(KCH)]
        w1_t = [[load_r(t["w1"][k * 128:(k + 1) * 128, m * 128:(m + 1) * 128], 128, 128, f"w1{k}{m}")
                 for m in range(4)] for k in range(4)]
        w2_t = [[load_r(t["w2"][k * 128:(k + 1) * 128, m * 128:(m + 1) * 128], 128, 128, f"w2{k}{m}")
                 for m in range(2)] for k in range(4)]
        w1bar_t = [load_r(t["w1bar"][k * 128:(k + 1) * 128, :], 128, 1, f"w1b{k}") for k in range(4)]
        ones_t = load_r(t["ones"], 128, 1, "ones")
        ident_t = load_r(t["ident"], 128, 128, "ident")
        bqk_t = load_f(t["bqk"], 128, 1, "bqk")
        bv_t = load_f(t["bv"], 128, 1, "bv")
        bo_t = [load_f(t["bo"][m * 128:(m + 1) * 128, :], 128, 1, f"bo{m}") for m in range(2)]
        b1_t = [load_f(t["b1"][m * 128:(m + 1) * 128, :], 128, 1, f"b1_{m}") for m in range(4)]
        b1bar_t = load_f(t["b1bar"], 1, 1, "b1bar")
        lng_t = [load_f(t["lng"][m * 128:(m + 1) * 128, :], 128, 1, f"lng{m}") for m in range(4)]
        lnb_t = [load_f(t["lnb"][m * 128:(m + 1) * 128, :], 128, 1, f"lnb{m}") for m in range(4)]
        b2_t = [load_f(t["b2"][m * 128:(m + 1) * 128, :], 128, 1, f"b2_{m}") for m in range(2)]

        xffn_r = [[None, None], [None, None]]
        xffn_f = [[None, None], [None, None]]
        for d in range(2):
            for k in range(KCH):
                sl = t["xf_r"][d][k * 128:(k + 1) * 128, :]
                xffn_r[d][k] = load_r(sl, 128, TH, f"xfr{d}{k}")
                xffn_f[d][k] = load_f(sl, 128, TH, f"xff{d}{k}")

        # ================= projections =================
        qkT = [None, None]   # [128(2h*64d), NT] fp32r
        v_t = [[None] * 16, [None] * 16]   # 16 x [128 tok, 128(2h*64d)]
        rows = es.enter_context(tc.tile_pool(name="rows", bufs=1))
        attn_es = ExitStack()
        qp = attn_es.enter_context(tc.tile_pool(name="qkv", bufs=1))
        with tc.tile_pool(name="xfull", bufs=1) as xp, \
             tc.tile_pool(name="qkps", bufs=1, space="PSUM") as qkps, \
             tc.tile_pool(name="vps", bufs=3, space="PSUM") as vps:
            xt = [[None, None], [None, None]]
            for s, src in enumerate((t["x0t"], t["x1t"])):
                for k in range(KCH):
                    xt[s][k] = xp.tile([128, NT], f32r, tag=f"x{s}{k}", name=f"x{s}{k}")
                    nc.gpsimd.dma_start(xt[s][k][:], src[k * 128:(k + 1) * 128, :])
            for s in range(2):
                ps = qkps.tile([128, NT], f32)
                for jn in range(NT // 512):
                    for k in range(KCH):
                        nc.tensor.matmul(ps[:, jn * 512:(jn + 1) * 512],
                                         wqk_t[k][:], xt[s][k][:, jn * 512:(jn + 1) * 512],
                                         start=(k == 0), stop=(k == KCH - 1))
                qkT[s] = qp.tile([128, NT], f32r, tag=f"qkT{s}", name=f"qkT{s}")
                nc.scalar.activation(qkT[s][:], ps[:], AF.Identity, bias=bqk_t[:], scale=1.0)
            for s in range(2):
                for it in range(16):
                    pv = vps.tile([128, 256], f32, tag="vps")
                    for var in range(2):
                        for k in range(KCH):
                            nc.tensor.matmul(pv[:, var * 128:(var + 1) * 128],
                                             xt[s][k][:, it * 128:(it + 1) * 128],
                                             wv_t[k][:, var * 128:(var + 1) * 128],
                                             start=(k == 0), stop=(k == KCH - 1))
                    v_t[s][it] = qp.tile([128, 256], f32r, tag=f"v{s}_{it}", name=f"v{s}_{it}")
                    nc.vector.tensor_copy(v_t[s][it][:], pv[:])

        # ================= attention (two symmetric paths) =================
        # path p: (A,B) = (p, 1-p); output = m for dir (1-p) tokens of x_{1-p}
        mn_pool = attn_es.enter_context(tc.tile_pool(name="mnorm", bufs=1))
        rsj = [rows.tile([128, 64], f32, tag=f"rsj{jj}", name=f"rsj{jj}") for jj in range(2)]
        # row-vector tiles: engine ops need base partition 0 (and equal
        # bases across SBUF operands), so each row vector gets its own tile
        m_raw = [None, None]
        mrp = attn_es.enter_context(tc.tile_pool(name="mraw", bufs=1))
        with tc.tile_pool(name="estrip", bufs=4) as ep, \
             tc.tile_pool(name="simps", bufs=3, space="PSUM") as simps, \
             tc.tile_pool(name="avps", bufs=2, space="PSUM") as avps:
            for p in range(2):
                A, Bi = p, 1 - p
                m_raw[p] = mrp.tile([128, NT], f32, tag=f"mraw{p}", name=f"mraw{p}")
                for jj in range(2):
                    av = [avps.tile([128, 512], f32, tag="av", name=f"av{p}_{jj}_{_i}") for _i in range(2)]
                    for it in range(16):
                        est = [None, None]
                        for h in range(2):
                            sp = simps.tile([128, 1024], f32, tag="sim")
                            for jc in range(2):
                                nc.tensor.matmul(
                                    sp[:, jc * 512:(jc + 1) * 512],
                                    qkT[A][64 * h:64 * (h + 1), it * 128:(it + 1) * 128],
                                    qkT[Bi][64 * h:64 * (h + 1),
                                            jj * 1024 + jc * 512:jj * 1024 + (jc + 1) * 512],
                                    start=True, stop=True,
                                    tile_position=(64 * h, 0))
                            est[h] = ep.tile([128, 1024], f32r, tag="est", name=f"est{h}")
                            col = (p * 2 + h) * 16 + it
                            nc.scalar.activation(est[h][:], sp[:], AF.Exp,
                                                 accum_out=rsj[jj][:, col:col + 1])
                        for jc in range(2):
                            for h in range(2):
                                # lhsT = zero-padded v variant h: rows 64h:64h+64
                                # of the product get head h's AV, rest zeros
                                nc.tensor.matmul(
                                    av[jc][:],
                                    v_t[A][it][:, h * 128:(h + 1) * 128],
                                    est[h][:, jc * 512:(jc + 1) * 512],
                                    start=(it == 0 and h == 0),
                                    stop=(it == 15 and h == 1))
                    for jc in range(2):
                        nc.vector.tensor_copy(
                            m_raw[p][:, jj * 1024 + jc * 512:jj * 1024 + (jc + 1) * 512],
                            av[jc][:])

        # ---- denominators: den[path p] = rowsums of path (1-p) ----
        rsall = rows.tile([128, 64], f32, tag="rsall")
        nc.vector.tensor_add(rsall[:], rsj[0][:], rsj[1][:])
        rsall_r = rows.tile([128, 64], f32r, tag="rsallr")
        nc.vector.tensor_copy(rsall_r[:], rsall[:])
        with tc.tile_pool(name="trps", bufs=1, space="PSUM") as trps:
            tp = trps.tile([64, 128], f32)
            nc.tensor.matmul(tp[:], rsall_r[:], ident_t[:], start=True, stop=True)
            rsT = rows.tile([64, 128], f32, tag="rsT")
            nc.vector.tensor_copy(rsT[:], tp[:])
        den_dram = dram.tile([4, 2048], f32)
        for r in range(4):
            nc.sync.dma_start(
                den_dram[r].rearrange("(it p) -> it p", it=16),
                rsT[r * 16:(r + 1) * 16, :])
        den_rows = mn_pool.tile([4, 2048], f32, tag="denrows", name="denrows")
        nc.sync.dma_start(den_rows[:], den_dram[:])
        lnden = mn_pool.tile([4, 2048], f32, tag="lnden", name="lnden")
        nc.scalar.activation(lnden[:], den_rows[:], AF.Ln)
        recipden = mn_pool.tile([4, 2048], f32, tag="recipden", name="recipden")
        nc.scalar.activation(recipden[:], lnden[:], AF.Exp, scale=-1.0)
        recip_dram = dram.tile([4, 2048], f32)
        nc.sync.dma_start(recip_dram[:], recipden[:])

        # ---- normalize + bv;  den for path p = rows (1-p)*2+h ----
        # DVE cannot take 0-step partition APs, so materialize the row
        # broadcast with a DMA from DRAM (partitions 64h:64h+64 <- head h row).
        m_norm = [None, None]
        for p in range(2):
            recipb = mn_pool.tile([128, NT], f32, tag=f"recipb{p}", name=f"recipb{p}")
            for h in range(2):
                r = (1 - p) * 2 + h
                nc.sync.dma_start(recipb[64 * h:64 * (h + 1), :],
                                  recip_dram[r:r + 1, :].to_broadcast((64, NT)))
            m_norm[p] = mn_pool.tile([128, NT], f32, tag=f"mnorm{p}", name=f"mnorm{p}")
            nc.vector.tensor_mul(m_norm[p][:], m_raw[p][:], recipb[:])
            nc.vector.tensor_scalar(m_norm[p][:], m_norm[p][:], bv_t[:], None, OP.add)

        # ======== exchange: 8-way AllToAll, reshard (b,hg) -> token-eighth ====
        # block r (of 8) = token columns [r*256:(r+1)*256]; after the
        # exchange, core c holds m for ALL batches at ITS 256-token slice.
        # bounce layout: [8 blocks, 2 paths, 128, 256]
        bounce_in = dram.tile([4, 2, 2, 128, 256], f32)   # (b_blk, kc_blk, path, p, t)
        bounce_out = dram.tile([4, 2, 2, 128, 256], f32)
        for p in range(2):
            for bb in range(4):
                for kb in range(2):
                    nc.sync.dma_start(
                        bounce_in[bb, kb, p],
                        m_norm[p][:, (2 * bb + kb) * 256:(2 * bb + kb + 1) * 256])
        nc.gpsimd.collective_compute(
            "AllToAll", mybir.AluOpType.bypass,
            replica_groups=[list(range(8))],
            ins=[bounce_in.opt()], outs=[bounce_out.opt()])
        attn_es.close()   # frees qkT/v/m_raw/m_norm SBUF for the FFN phase
        # out block s = from core s=(b=s//2, hg=s%2): m[batch b, heads hg, my toks]
        # m_dir[d][kc][:, b*256:(b+1)*256] = bounce_out[2b+kc, 1-d]
        mdir = [[None, None], [None, None]]   # [dir][kc] -> [128, TH=4x256] f32r
        mpool = es.enter_context(tc.tile_pool(name="mdir", bufs=1))
        for d in range(2):
            p = 1 - d
            for kc in range(2):
                mdir[d][kc] = mpool.tile([128, TH], f32r, tag=f"mdir{d}{kc}", name=f"mdir{d}{kc}")
                for bb in range(4):
                    nc.gpsimd.dma_start(mdir[d][kc][:, bb * 256:(bb + 1) * 256],
                                        bounce_out[bb, kc, p])

        # ================= out-projection =================
        mproj = [[None, None], [None, None]]
        with tc.tile_pool(name="mpps", bufs=2, space="PSUM") as mpps:
            for d in range(2):
                for mo in range(2):
                    ps = mpps.tile([128, TH], f32, tag="mp")
                    for nn in range(2):
                        for kc in range(2):
                            nc.tensor.matmul(ps[:, nn * 512:(nn + 1) * 512],
                                             wo_t[kc][mo][:],
                                             mdir[d][kc][:, nn * 512:(nn + 1) * 512],
                                             start=(kc == 0), stop=(kc == 1))
                    mproj[d][mo] = mpool.tile([128, TH], f32r, tag=f"mproj{d}{mo}", name=f"mproj{d}{mo}")
                    nc.scalar.activation(mproj[d][mo][:], ps[:], AF.Identity,
                                         bias=bo_t[mo][:], scale=1.0)

        # ================= FFN =================
        # ccT chunks (f32r): [xffn_r[d][0], xffn_r[d][1], mproj[d][0], mproj[d][1]]
        hsb_pool = es.enter_context(tc.tile_pool(name="hsb", bufs=1))
        hsb = {}
        statp = es.enter_context(tc.tile_pool(name="statrows", bufs=1))
        mu_all = statp.tile([1, 2048], f32, tag="muall", name="muall")
        ss_all = statp.tile([1, 2048], f32, tag="srowA", name="ssall")
        with tc.tile_pool(name="sq", bufs=3) as sqp, \
             tc.tile_pool(name="hps", bufs=4, space="PSUM") as hps, \
             tc.tile_pool(name="rowps", bufs=2, space="PSUM") as rowps:
            for d in range(2):
                cc = [xffn_r[d][0], xffn_r[d][1], mproj[d][0], mproj[d][1]]
                for tcn in range(2):
                    sl = slice(tcn * 512, (tcn + 1) * 512)
                    col = (d * 2 + tcn) * 512
                    pmu = rowps.tile([1, 512], f32, tag="pmu")
                    for kc in range(4):
                        nc.tensor.matmul(pmu[:], w1bar_t[kc][:], cc[kc][:, sl],
                                         start=(kc == 0), stop=(kc == 3))
                    nc.vector.tensor_scalar(mu_all[0:1, col:col + 512], pmu[:],
                                            b1bar_t[:], None, OP.add)
                    pss = rowps.tile([1, 512], f32, tag="pss")
                    for mh in range(4):
                        ph = hps.tile([128, 512], f32, tag="ph")
                        for kc in range(4):
                            nc.tensor.matmul(ph[:], w1_t[kc][mh][:], cc[kc][:, sl],
                                             start=(kc == 0), stop=(kc == 3))
                        hkey = (d, tcn, mh)
                        hsb[hkey] = hsb_pool.tile([128, 512], f32, tag=f"h{d}{tcn}{mh}", name=f"h{d}{tcn}{mh}")
                        nc.vector.tensor_scalar(hsb[hkey][:], ph[:], b1_t[mh][:],
                                                None, OP.add)
                        sq = sqp.tile([128, 512], f32r, tag="sq")
                        nc.vector.tensor_mul(sq[:], hsb[hkey][:], hsb[hkey][:])
                        nc.tensor.matmul(pss[:], ones_t[:], sq[:],
                                         start=(mh == 0), stop=(mh == 3))
                    nc.vector.tensor_copy(ss_all[0:1, col:col + 512], pss[:])

        # batched LN stats: rstd = exp(-0.5 ln(ss/512 - mu^2 + eps))
        musq = statp.tile([1, 2048], f32, tag="srowB", name="musq")
        nc.vector.tensor_mul(musq[:], mu_all[:], mu_all[:])
        ve = statp.tile([1, 2048], f32, tag="srowC", name="ve")
        nc.vector.scalar_tensor_tensor(ve[:], ss_all[:], 1.0 / HID, musq[:],
                                       OP.mult, OP.subtract)
        vee = statp.tile([1, 2048], f32, tag="srowA", name="vee")
        nc.vector.tensor_scalar(vee[:], ve[:], LN_EPS, None, OP.add)
        lnve = statp.tile([1, 2048], f32, tag="srowB", name="lnve")
        nc.scalar.activation(lnve[:], vee[:], AF.Ln)
        rstd = statp.tile([1, 2048], f32, tag="srowA", name="rstd")
        nc.scalar.activation(rstd[:], lnve[:], AF.Exp, scale=-0.5)
        murstd = statp.tile([1, 2048], f32, tag="srowB", name="murstd")
        nc.vector.tensor_mul(murstd[:], mu_all[:], rstd[:])
        # materialize partition-broadcasts of rstd/murstd via DRAM
        stat_dram = dram.tile([2, 2048], f32)
        nc.sync.dma_start(stat_dram[0:1, :], rstd[:])
        nc.sync.dma_start(stat_dram[1:2, :], murstd[:])
        statb = es.enter_context(tc.tile_pool(name="statb", bufs=1))
        rstdb = statb.tile([128, 2048], f32, tag="rstdb", name="rstdb")
        murstdb = statb.tile([128, 2048], f32, tag="murstdb", name="murstdb")
        nc.sync.dma_start(rstdb[:], stat_dram[0:1, :].to_broadcast((128, 2048)))
        nc.sync.dma_start(murstdb[:], stat_dram[1:2, :].to_broadcast((128, 2048)))

        # affine + gelu + W2 + residual
        with tc.tile_pool(name="uacts", bufs=3) as up, \
             tc.tile_pool(name="gacts", bufs=5) as gp, \
             tc.tile_pool(name="osb", bufs=4) as op_, \
             tc.tile_pool(name="ops", bufs=2, space="PSUM") as ops:
            for d in range(2):
                for tcn in range(2):
                    sl = slice(tcn * 512, (tcn + 1) * 512)
                    col = (d * 2 + tcn) * 512
                    rsl = rstdb[:, col:col + 512]
                    msl = murstdb[:, col:col + 512]
                    gh = [None] * 4
                    for mh in range(4):
                        u = up.tile([128, 512], f32, tag="u")
                        nc.vector.tensor_mul(u[:], hsb[(d, tcn, mh)][:], rsl[:])
                        t2 = up.tile([128, 512], f32, tag="t2")
                        nc.vector.tensor_sub(t2[:], u[:], msl[:])
                        gh[mh] = gp.tile([128, 512], f32r, tag="gh", name=f"gh{mh}")
                        nc.scalar.activation(gh[mh][:], t2[:], AF.Gelu,
                                             bias=lnb_t[mh][:], scale=lng_t[mh][:])
                    for mo in range(2):
                        po = ops.tile([128, 512], f32, tag="po")
                        for kh in range(4):
                            nc.tensor.matmul(po[:], w2_t[kh][mo][:], gh[kh][:],
                                             start=(kh == 0), stop=(kh == 3))
                        ot = op_.tile([128, 512], f32, tag="ot")
                        nc.vector.scalar_tensor_tensor(
                            ot[:], po[:], b2_t[mo][:], xffn_f[d][mo][:, sl],
                            OP.add, OP.add)
                        nc.sync.dma_start(t["outs"][d][mo * 128:(mo + 1) * 128, sl], ot[:])


def _host_prep(inputs):
    """Build per-core in_maps from full inputs."""
    x0 = np.asarray(inputs["x0"], np.float32)
    x1 = np.asarray(inputs["x1"], np.float32)
    Wqk = np.asarray(inputs["Wqk"], np.float32) * (D ** -0.25)
    bqk = np.asarray(inputs["bqk"], np.float32) * (D ** -0.25)
    Wv = np.asarray(inputs["Wv"], np.float32)
    bv = np.asarray(inputs["bv"], np.float32)
    Wo = np.asarray(inputs["Wo"], np.float32)
    bo = np.asarray(inputs["bo"], np.float32)
    W1 = np.asarray(inputs["W1"], np.float32)
    b1 = np.asarray(inputs["b1"], np.float32)
    lng = np.asarray(inputs["ln_g"], np.float32)
    lnb = np.asarray(inputs["ln_b"], np.float32)
    W2 = np.asarray(inputs["W2"], np.float32)
    b2 = np.asarray(inputs["b2"], np.float32)

    shared = {
        "wo": np.ascontiguousarray(Wo),
        "bo": bo.reshape(E, 1),
        "w1": np.ascontiguousarray(W1),
        "w1bar": W1.mean(axis=1).reshape(HID, 1),
        "b1": b1.reshape(HID, 1),
        "b1bar": np.array([[b1.mean()]], np.float32),
        "lng": lng.reshape(HID, 1),
        "lnb": lnb.reshape(HID, 1),
        "w2": np.ascontiguousarray(W2),
        "b2": b2.reshape(E, 1),
        "ident": np.eye(128, dtype=np.float32),
        "ones": np.ones((128, 1), np.float32),
    }
    in_maps = []
    for c in range(N_CORES):
        b, hg = c // 2, c % 2
        hs = slice(hg * 128, hg * 128 + 128)
        ts = slice(hg * TH, hg * TH + TH)
        m = dict(shared)
        m["x0t"] = np.ascontiguousarray(x0[b].T)
        m["x1t"] = np.ascontiguousarray(x1[b].T)
        # FFN slice: my 256-token slice of EVERY batch, columns (b, t) b-major
        cs = slice(c * 256, (c + 1) * 256)
        m["x0t_ffn"] = np.ascontiguousarray(
            x0[:, cs, :].reshape(B * 256, E).T)
        m["x1t_ffn"] = np.ascontiguousarray(
            x1[:, cs, :].reshape(B * 256, E).T)
        m["wqk"] = np.ascontiguousarray(Wqk[:, hs])
        m["bqk"] = bqk[hs].reshape(128, 1)
        wvp = np.zeros((E, 256), np.float32)
        wvp[:, 0:64] = Wv[:, hg * 128:hg * 128 + 64]        # head0 -> cols 0:64
        wvp[:, 192:256] = Wv[:, hg * 128 + 64:hg * 128 + 128]  # head1 -> cols 192:256
        m["wv"] = wvp
        m["bv"] = bv[hs].reshape(128, 1)
        in_maps.append(m)
    return in_maps


def _get_nc():
    if "nc" not in _cache:
        _cache["nc"] = _build()
    return _cache["nc"]


def kernel(**inputs):
    from concourse import bass_utils
    nc = _get_nc()
    in_maps = _host_prep(inputs)
    res = bass_utils.run_bass_kernel_spmd(nc, in_maps, core_ids=list(range(N_CORES)))
    out0 = np.empty((B, NT, E), np.float32)
    out1 = np.empty((B, NT, E), np.float32)
    for c in range(N_CORES):
        cs = slice(c * 256, (c + 1) * 256)
        o0 = res.results[c]["out0t"]  # [E, 4*256], cols (b, t)
        o1 = res.results[c]["out1t"]
        for b in range(B):
            out0[b, cs, :] = o0[:, b * 256:(b + 1) * 256].T
            out1[b, cs, :] = o1[:, b * 256:(b + 1) * 256].T
    return out0, out1



# revision 4
# speedup vs baseline: 1.3579x; 1.3579x over previous
"""CrossTransformer Trainium2 kernel — 8 NeuronCores (bf16 compute).

Sharding: core c = (batch b = c//2, head-pair hg = c%2).  Attention is
head-parallel (2 heads/core, bf16 matmuls, exp on ACT with fused
row-sum accum); out-proj + FFN are token-parallel (a 256-token slice of
every batch per core) after an 8-way AllToAll of the attention output.

Both softmax directions run the same "column-softmax" pipeline with
swapped inputs; each path's softmax denominator is the opposite path's
exp row-sum (E_ba = E_ab^T).

Bias folding (host side): bv folds through Wo into bo
(bo_eff = bo + bv@Wo), and bo_eff folds through W1 into b1
(b1_eff = b1 + bo_eff@W1[E:]), so the v-proj and out-proj carry no
bias at all on device.
"""
import numpy as np

B, NT, E, H, D = 4, 2048, 256, 4, 64
HPC = 2            # heads per core
TH = NT // 2       # token half (per-core FFN token count = 4*256)
HID = 2 * E        # FFN hidden (512)
KCH = E // 128     # 128-chunks of E (2)
N_CORES = 8
LN_EPS = 1e-5

_cache = {}


def _build():
    import concourse.bass as bass
    import concourse.tile as tile
    from concourse import bacc
    import concourse.mybir as mybir

    dt = mybir.dt
    f32, bf16 = dt.float32, dt.bfloat16

    nc = bacc.Bacc("TRN2", target_bir_lowering=False, debug=False,
                   num_devices=N_CORES)

    def din(name, shape, dtype):
        return nc.dram_tensor(name, shape, dtype, kind="ExternalInput").ap()

    t = dict(
        x0t=din("x0t", [E, NT], bf16),          # x0[b].T
        x1t=din("x1t", [E, NT], bf16),
        xfb=[din(f"x{d}t_ffn", [E, TH], bf16) for d in (0, 1)],
        xff=[din(f"x{d}t_ffn32", [E, TH], f32) for d in (0, 1)],
        wqk=din("wqk", [E, 128], bf16),         # pre-scaled, this core's heads
        bqk=din("bqk", [128, 1], f32),
        wv=din("wv", [E, 256], bf16),           # head0->cols 0:64, head1->192:256
        wo=din("wo", [E, E], bf16),
        w1=din("w1", [HID, HID], bf16),
        w1bar=din("w1bar", [HID, 1], bf16),
        b1=din("b1", [HID, 1], f32),            # b1_eff
        b1bar=din("b1bar", [1, 1], f32),        # b1bar_eff
        lng=din("lng", [HID, 1], f32),
        lnb=din("lnb", [HID, 1], f32),
        w2=din("w2", [HID, E], bf16),
        b2=din("b2", [E, 1], f32),
        ident=din("ident", [128, 128], bf16),
        ones=din("ones", [128, 1], bf16),       # column of ones (ss rowsums)
        ones1=din("ones1", [1, 128], bf16),     # single-partition row of ones
        outs=[nc.dram_tensor(f"out{d}t", [E, TH], f32,
                             kind="ExternalOutput").ap() for d in (0, 1)],
    )

    with tile.TileContext(nc) as tc:
        _body(nc, tc, bass, mybir, tile, t)
    nc.compile()
    return nc


def _body(nc, tc, bass, mybir, tile, t):
    from contextlib import ExitStack
    dt = mybir.dt
    AF = mybir.ActivationFunctionType
    OP = mybir.AluOpType
    f32, bf16 = dt.float32, dt.bfloat16

    es = ExitStack()
    with es:
        wpool = es.enter_context(tc.tile_pool(name="weights", bufs=1))
        dram = es.enter_context(tc.tile_pool(name="dram", bufs=1, space="DRAM"))

        def load(ap_src, p, fshape, tag, dtype, eng):
            til = wpool.tile([p, fshape], dtype, tag=tag, name=tag)
            eng.dma_start(til[:], ap_src)
            return til

        S, G = nc.sync, nc.gpsimd
        # critical-path loads first (qk proj): wqk + x transposes
        wqk_t = [load(t["wqk"][k * 128:(k + 1) * 128, :], 128, 128, f"wqk{k}", bf16, S)
                 for k in range(KCH)]
        xt = [[None, None], [None, None]]
        for s, src in enumerate((t["x0t"], t["x1t"])):
            for k in range(KCH):
                xt[s][k] = wpool.tile([128, NT], bf16, tag=f"x{s}{k}", name=f"x{s}{k}")
                (S if s == 0 else G).dma_start(xt[s][k][:], src[k * 128:(k + 1) * 128, :])
        wv_t = [load(t["wv"][k * 128:(k + 1) * 128, :], 128, 256, f"wv{k}", bf16, S)
                for k in range(KCH)]
        bqk_t = load(t["bqk"], 128, 1, "bqk", f32, S)
        ident_t = load(t["ident"], 128, 128, "ident", bf16, G)
        ones_t = load(t["ones"], 128, 1, "ones", bf16, G)
        ones1_t = load(t["ones1"], 1, 128, "ones1", bf16, G)
        wo_t = [[load(t["wo"][k * 128:(k + 1) * 128, m * 128:(m + 1) * 128], 128, 128,
                      f"wo{k}{m}", bf16, G) for m in range(2)] for k in range(KCH)]
        w1_t = [[load(t["w1"][k * 128:(k + 1) * 128, m * 128:(m + 1) * 128], 128, 128,
                      f"w1{k}{m}", bf16, G) for m in range(4)] for k in range(4)]
        w2_t = [[load(t["w2"][k * 128:(k + 1) * 128, m * 128:(m + 1) * 128], 128, 128,
                      f"w2{k}{m}", bf16, G) for m in range(2)] for k in range(4)]
        w1bar_t = [load(t["w1bar"][k * 128:(k + 1) * 128, :], 128, 1, f"w1b{k}", bf16, G)
                   for k in range(4)]
        b1_t = [load(t["b1"][m * 128:(m + 1) * 128, :], 128, 1, f"b1_{m}", f32, G)
                for m in range(4)]
        b1bar_t = load(t["b1bar"], 1, 1, "b1bar", f32, G)
        lng_t = [load(t["lng"][m * 128:(m + 1) * 128, :], 128, 1, f"lng{m}", f32, G)
                 for m in range(4)]
        lnb_t = [load(t["lnb"][m * 128:(m + 1) * 128, :], 128, 1, f"lnb{m}", f32, G)
                 for m in range(4)]
        b2_t = [load(t["b2"][m * 128:(m + 1) * 128, :], 128, 1, f"b2_{m}", f32, G)
                for m in range(2)]
        xfb = [[load(t["xfb"][d][k * 128:(k + 1) * 128, :], 128, TH, f"xfb{d}{k}", bf16, G)
                for k in range(KCH)] for d in range(2)]
        xff = [[load(t["xff"][d][k * 128:(k + 1) * 128, :], 128, TH, f"xff{d}{k}", f32, G)
                for k in range(KCH)] for d in range(2)]

        # ================= projections =================
        qkT = [None, None]   # [128(2h*64d), NT] bf16
        v_t = [[None] * 16, [None] * 16]   # 16 x [128 tok, 256(h0|0|0|h1)] bf16
        rows = es.enter_context(tc.tile_pool(name="rows", bufs=1))
        attn_es = ExitStack()
        qp = attn_es.enter_context(tc.tile_pool(name="qkv", bufs=1))
        with tc.tile_pool(name="qkps", bufs=1, space="PSUM") as qkps, \
             tc.tile_pool(name="vps", bufs=3, space="PSUM") as vps:
            for s in range(2):
                ps = qkps.tile([128, NT], f32, tag="qkps")
                for jn in range(NT // 512):
                    for k in range(KCH):
                        nc.tensor.matmul(ps[:, jn * 512:(jn + 1) * 512],
                                         wqk_t[k][:], xt[s][k][:, jn * 512:(jn + 1) * 512],
                                         start=(k == 0), stop=(k == KCH - 1))
                qkT[s] = qp.tile([128, NT], bf16, tag=f"qkT{s}", name=f"qkT{s}")
                nc.vector.tensor_scalar(qkT[s][:], ps[:], bqk_t[:], None, OP.add)
            for s in range(2):
                for it in range(16):
                    pv = vps.tile([128, 256], f32, tag="vps")
                    for k in range(KCH):
                        nc.tensor.matmul(pv[:], xt[s][k][:, it * 128:(it + 1) * 128],
                                         wv_t[k][:], start=(k == 0), stop=(k == KCH - 1))
                    v_t[s][it] = qp.tile([128, 256], bf16, tag=f"v{s}_{it}", name=f"v{s}_{it}")
                    nc.vector.tensor_copy(v_t[s][it][:], pv[:])

        # ================= attention (two symmetric paths) =================
        # path p: (A,B) = (p, 1-p); est = exp(qkA^T qkB) [tokA, tokB];
        # m_raw[p][feat, tokB] = sum_tokA v_A est;  den = opposite path's accum
        mrp = attn_es.enter_context(tc.tile_pool(name="mraw", bufs=1))
        rsj = [rows.tile([128, 64], f32, tag=f"rsj{jj}", name=f"rsj{jj}")
               for jj in range(2)]
        m_raw = [None, None]
        with tc.tile_pool(name="estrip", bufs=4) as ep, \
             tc.tile_pool(name="simps", bufs=3, space="PSUM") as simps, \
             tc.tile_pool(name="avps", bufs=2, space="PSUM") as avps:
            for p in range(2):
                A, Bi = p, 1 - p
                m_raw[p] = mrp.tile([128, NT], bf16, tag=f"mraw{p}", name=f"mraw{p}")
                for jj in range(2):
                    av = [avps.tile([128, 512], f32, tag="av",
                                    name=f"av{p}_{jj}_{_i}") for _i in range(2)]
                    for it in range(16):
                        est = [None, None]
                        for h in range(2):
                            sp = simps.tile([128, 1024], f32, tag="sim")
                            for jc in range(2):
                                nc.tensor.matmul(
                                    sp[:, jc * 512:(jc + 1) * 512],
                                    qkT[A][64 * h:64 * (h + 1), it * 128:(it + 1) * 128],
                                    qkT[Bi][64 * h:64 * (h + 1),
                                            jj * 1024 + jc * 512:jj * 1024 + (jc + 1) * 512],
                                    start=True, stop=True,
                                    tile_position=(64 * h, 0))
                            est[h] = ep.tile([128, 1024], bf16, tag="est", name=f"est{h}")
                            col = (p * 2 + h) * 16 + it
                            nc.scalar.activation(est[h][:], sp[:], AF.Exp,
                                                 accum_out=rsj[jj][:, col:col + 1])
                        for h in range(2):
                            # lhsT = zero-padded v half h: product rows
                            # 64h:64h+64 get head h's AV, other 64 rows zeros
                            for jc in range(2):
                                nc.tensor.matmul(
                                    av[jc][:],
                                    v_t[A][it][:, h * 128:(h + 1) * 128],
                                    est[h][:, jc * 512:(jc + 1) * 512],
                                    start=(it == 0 and h == 0),
                                    stop=(it == 15 and h == 1))
                    for jc in range(2):
                        nc.vector.tensor_copy(
                            m_raw[p][:, jj * 1024 + jc * 512:jj * 1024 + (jc + 1) * 512],
                            av[jc][:])

        # ---- denominators: recip of row sums, transposed + broadcast ----
        rsall = rows.tile([128, 64], f32, tag="rsall")
        nc.vector.tensor_add(rsall[:], rsj[0][:], rsj[1][:])
        recip = rows.tile([128, 64], f32, tag="recip")
        nc.vector.reciprocal(recip[:], rsall[:])
        recip_bf = rows.tile([128, 64], bf16, tag="recipbf")
        nc.vector.tensor_copy(recip_bf[:], recip[:])
        with tc.tile_pool(name="trps", bufs=1, space="PSUM") as trps:
            tp = trps.tile([64, 128], f32)
            nc.tensor.matmul(tp[:], recip_bf[:], ident_t[:], start=True, stop=True)
            rsT = rows.tile([64, 128], bf16, tag="rsT")
            nc.vector.tensor_copy(rsT[:], tp[:])
        # row r of recip_dram = 1/den for (p=r//2, h=r%2) over all 2048 toks
        recip_dram = dram.tile([4, 2048], bf16)
        for r in range(4):
            nc.sync.dma_start(
                recip_dram[r].rearrange("(it t) -> it t", it=16),
                rsT[r * 16:(r + 1) * 16, :])
        recipd = [rows.tile([1, 2048], bf16, tag=f"recipd{r}", name=f"recipd{r}")
                  for r in range(4)]
        for r in range(4):
            nc.sync.dma_start(recipd[r][:], recip_dram[r:r + 1, :])

        # bounce layout: [8 dest-core blocks, 2 paths, 128, 256] bf16
        bounce_in = dram.tile([8, 2, 128, 256], bf16)
        bounce_out = dram.tile([8, 2, 128, 256], bf16)
        m_nb = [None, None]
        with tc.tile_pool(name="rbps", bufs=2, space="PSUM") as rbps:
            for p in range(2):
                rb = rbps.tile([128, NT], f32, tag="rb")
                for h in range(2):
                    r = (1 - p) * 2 + h
                    for nn in range(4):
                        nc.tensor.matmul(
                            rb[64 * h:64 * (h + 1), nn * 512:(nn + 1) * 512],
                            ones1_t[0:1, 0:64],
                            recipd[r][0:1, nn * 512:(nn + 1) * 512],
                            start=True, stop=True,
                            tile_position=(0, 64 * h))
                m_nb[p] = mrp.tile([128, NT], bf16, tag=f"mnb{p}", name=f"mnb{p}")
                nc.vector.tensor_mul(m_nb[p][:], m_raw[p][:], rb[:])
                for j in range(8):
                    (nc.sync if j % 2 == 0 else nc.gpsimd).dma_start(
                        bounce_in[j, p], m_nb[p][:, j * 256:(j + 1) * 256])

        nc.gpsimd.collective_compute(
            "AllToAll", mybir.AluOpType.bypass,
            replica_groups=[list(range(8))],
            ins=[bounce_in.opt()], outs=[bounce_out.opt()])
        attn_es.close()   # frees qkT/v/m_raw SBUF for the FFN phase

        # ---- FFN x-part, runs during the collective (no dependency) ----
        hx = {}
        mux = {}
        hsb_pool = es.enter_context(tc.tile_pool(name="hsb", bufs=1))
        statp = es.enter_context(tc.tile_pool(name="statrows", bufs=1))
        with tc.tile_pool(name="hxps", bufs=3, space="PSUM") as hxps, \
             tc.tile_pool(name="rowxps", bufs=2, space="PSUM") as rowxps:
            for d in range(2):
                for tcn in range(2):
                    sl = slice(tcn * 512, (tcn + 1) * 512)
                    pmu = rowxps.tile([1, 512], f32, tag="pmux")
                    for kc in range(KCH):
                        nc.tensor.matmul(pmu[:], w1bar_t[kc][:], xfb[d][kc][:, sl],
                                         start=(kc == 0), stop=(kc == KCH - 1))
                    mux[(d, tcn)] = statp.tile([1, 512], f32, tag=f"mux{d}{tcn}",
                                               name=f"mux{d}{tcn}")
                    nc.vector.tensor_scalar(mux[(d, tcn)][:], pmu[:], b1bar_t[:],
                                            None, OP.add)
                    for mh in range(4):
                        ph = hxps.tile([128, 512], f32, tag="phx")
                        for kc in range(KCH):
                            nc.tensor.matmul(ph[:], w1_t[kc][mh][:], xfb[d][kc][:, sl],
                                             start=(kc == 0), stop=(kc == KCH - 1))
                        key = (d, tcn, mh)
                        hx[key] = hsb_pool.tile([128, 512], bf16, tag=f"hx{d}{tcn}{mh}",
                                                name=f"hx{d}{tcn}{mh}")
                        nc.vector.tensor_scalar(hx[key][:], ph[:], b1_t[mh][:],
                                                None, OP.add)

        # ---- gather exchanged m; out-projection (no bias: folded) ----
        mpool = es.enter_context(tc.tile_pool(name="mdir", bufs=1))
        mdir = [[None, None], [None, None]]
        for d in range(2):
            p = 1 - d
            for kc in range(2):
                mdir[d][kc] = mpool.tile([128, TH], bf16, tag=f"mdir{d}{kc}",
                                         name=f"mdir{d}{kc}")
                for bb in range(4):
                    (nc.sync if bb % 2 == 0 else nc.gpsimd).dma_start(
                        mdir[d][kc][:, bb * 256:(bb + 1) * 256],
                        bounce_out[bb * 2 + kc, p])
        mproj = [[None, None], [None, None]]
        with tc.tile_pool(name="mpps", bufs=2, space="PSUM") as mpps:
            for d in range(2):
                for mo in range(2):
                    po = mpps.tile([128, TH], f32, tag="mp")
                    for nn in range(2):
                        for kc in range(2):
                            nc.tensor.matmul(po[:, nn * 512:(nn + 1) * 512],
                                             wo_t[kc][mo][:],
                                             mdir[d][kc][:, nn * 512:(nn + 1) * 512],
                                             start=(kc == 0), stop=(kc == 1))
                    mproj[d][mo] = mpool.tile([128, TH], bf16, tag=f"mproj{d}{mo}",
                                              name=f"mproj{d}{mo}")
                    nc.vector.tensor_copy(mproj[d][mo][:], po[:])

        # ================= FFN m-part + LN stats =================
        hsb = {}
        t2 = {}
        bcast = {}
        with tc.tile_pool(name="sq", bufs=3) as sqp, \
             tc.tile_pool(name="t2p", bufs=1) as t2p, \
             tc.tile_pool(name="hps", bufs=2, space="PSUM") as hps, \
             tc.tile_pool(name="rowps", bufs=1, space="PSUM") as rowps, \
             tc.tile_pool(name="bps", bufs=1, space="PSUM") as bps, \
             tc.tile_pool(name="gacts", bufs=1) as gp, \
             tc.tile_pool(name="osb", bufs=4) as op_, \
             tc.tile_pool(name="ops", bufs=2, space="PSUM") as ops:
            for d in range(2):
                for tcn in range(2):
                    sl = slice(tcn * 512, (tcn + 1) * 512)
                    pmu = rowps.tile([1, 512], f32, tag="pmu")
                    for kc in range(2):
                        nc.tensor.matmul(pmu[:], w1bar_t[2 + kc][:],
                                         mproj[d][kc][:, sl],
                                         start=(kc == 0), stop=(kc == 1))
                    mu_row = statp.tile([1, 512], f32, tag=f"mu{d}{tcn}",
                                        name=f"mu{d}{tcn}")
                    nc.vector.tensor_add(mu_row[:], pmu[:], mux[(d, tcn)][:])
                    pss = rowps.tile([1, 512], f32, tag="pss")
                    for mh in range(4):
                        ph = hps.tile([128, 512], f32, tag="ph")
                        for kc in range(2):
                            nc.tensor.matmul(ph[:], w1_t[2 + kc][mh][:],
                                             mproj[d][kc][:, sl],
                                             start=(kc == 0), stop=(kc == 1))
                        key = (d, tcn, mh)
                        hsb[key] = hsb_pool.tile([128, 512], bf16, tag=f"h{d}{tcn}{mh}",
                                                 name=f"h{d}{tcn}{mh}")
                        nc.vector.tensor_add(hsb[key][:], ph[:], hx[key][:])
                        sq = sqp.tile([128, 512], bf16, tag="sq")
                        nc.vector.tensor_mul(sq[:], hsb[key][:], hsb[key][:])
                        nc.tensor.matmul(pss[:], ones_t[:], sq[:],
                                         start=(mh == 0), stop=(mh == 3))
                    # var = ss/HID - mu^2 ; rstd = exp(-0.5*ln(var + eps))
                    musq = statp.tile([1, 512], f32, tag="musq")
                    nc.vector.tensor_mul(musq[:], mu_row[:], mu_row[:])
                    var = statp.tile([1, 512], f32, tag="var")
                    nc.vector.scalar_tensor_tensor(var[:], pss[:], 1.0 / HID,
                                                   musq[:], OP.mult, OP.subtract)
                    nc.vector.tensor_scalar(var[:], var[:], LN_EPS, None, OP.add)
                    lnv = statp.tile([1, 512], f32, tag="lnv")
                    nc.scalar.activation(lnv[:], var[:], AF.Ln)
                    rstd = statp.tile([1, 512], f32, tag="rstd")
                    nc.scalar.activation(rstd[:], lnv[:], AF.Exp, scale=-0.5)
                    murstd = statp.tile([1, 512], f32, tag="murstd")
                    nc.vector.tensor_mul(murstd[:], mu_row[:], rstd[:])
                    rstd_bf = statp.tile([1, 512], bf16, tag="rstdbf")
                    nc.vector.tensor_copy(rstd_bf[:], rstd[:])
                    murstd_bf = statp.tile([1, 512], bf16, tag="murstdbf")
                    nc.vector.tensor_copy(murstd_bf[:], murstd[:])
                    rstdb = bps.tile([128, 512], f32, tag="rstdb")
                    nc.tensor.matmul(rstdb[:], ones1_t[:], rstd_bf[:],
                                     start=True, stop=True)
                    murstdb = bps.tile([128, 512], f32, tag="murstdb")
                    nc.tensor.matmul(murstdb[:], ones1_t[:], murstd_bf[:],
                                     start=True, stop=True)
                    for mh in range(4):
                        key = (d, tcn, mh)
                        u = sqp.tile([128, 512], bf16, tag="sq")
                        nc.vector.tensor_mul(u[:], hsb[key][:], rstdb[:])
                        t2[key] = t2p.tile([128, 512], bf16, tag=f"t2{d}{tcn}{mh}",
                                           name=f"t2{d}{tcn}{mh}")
                        nc.vector.tensor_sub(t2[key][:], u[:], murstdb[:])

            # all gelus together (one ACT table-set switch), then W2+residual
            gh = {}
            for d in range(2):
                for tcn in range(2):
                    for mh in range(4):
                        key = (d, tcn, mh)
                        gh[key] = gp.tile([128, 512], bf16, tag=f"gh{d}{tcn}{mh}",
                                          name=f"gh{d}{tcn}{mh}")
                        nc.scalar.activation(gh[key][:], t2[key][:], AF.Gelu,
                                             bias=lnb_t[mh][:], scale=lng_t[mh][:])
            for d in range(2):
                for tcn in range(2):
                    sl = slice(tcn * 512, (tcn + 1) * 512)
                    for mo in range(2):
                        po = ops.tile([128, 512], f32, tag="po")
                        for kh in range(4):
                            nc.tensor.matmul(po[:], w2_t[kh][mo][:],
                                             gh[(d, tcn, kh)][:],
                                             start=(kh == 0), stop=(kh == 3))
                        ot = op_.tile([128, 512], f32, tag="ot")
                        nc.vector.scalar_tensor_tensor(
                            ot[:], po[:], b2_t[mo][:], xff[d][mo][:, sl],
                            OP.add, OP.add)
                        nc.sync.dma_start(t["outs"][d][mo * 128:(mo + 1) * 128, sl],
                                          ot[:])


def _host_prep(inputs):
    """Build per-core in_maps from full inputs (bf16 matmul operands)."""
    import ml_dtypes
    bf16 = ml_dtypes.bfloat16
    x0 = np.asarray(inputs["x0"], np.float32)
    x1 = np.asarray(inputs["x1"], np.float32)
    Wqk = np.asarray(inputs["Wqk"], np.float32) * (D ** -0.25)
    bqk = np.asarray(inputs["bqk"], np.float32) * (D ** -0.25)
    Wv = np.asarray(inputs["Wv"], np.float32)
    bv = np.asarray(inputs["bv"], np.float32)
    Wo = np.asarray(inputs["Wo"], np.float32)
    bo = np.asarray(inputs["bo"], np.float32)
    W1 = np.asarray(inputs["W1"], np.float32)
    b1 = np.asarray(inputs["b1"], np.float32)
    lng = np.asarray(inputs["ln_g"], np.float32)
    lnb = np.asarray(inputs["ln_b"], np.float32)
    W2 = np.asarray(inputs["W2"], np.float32)
    b2 = np.asarray(inputs["b2"], np.float32)

    # fold bv through Wo, then bo_eff through W1's m-half
    bo_eff = bo + bv @ Wo
    b1_eff = b1 + bo_eff @ W1[E:, :]

    shared = {
        "wo": Wo.astype(bf16),
        "w1": W1.astype(bf16),
        "w1bar": W1.mean(axis=1).reshape(HID, 1).astype(bf16),
        "b1": b1_eff.reshape(HID, 1),
        "b1bar": np.array([[b1_eff.mean()]], np.float32),
        "lng": lng.reshape(HID, 1),
        "lnb": lnb.reshape(HID, 1),
        "w2": W2.astype(bf16),
        "b2": b2.reshape(E, 1),
        "ident": np.eye(128, dtype=bf16),
        "ones": np.ones((128, 1), bf16),
        "ones1": np.ones((1, 128), bf16),
    }
    in_maps = []
    for c in range(N_CORES):
        b, hg = c // 2, c % 2
        hs = slice(hg * 128, hg * 128 + 128)
        m = dict(shared)
        m["x0t"] = np.ascontiguousarray(x0[b].T).astype(bf16)
        m["x1t"] = np.ascontiguousarray(x1[b].T).astype(bf16)
        # FFN slice: my 256-token slice of EVERY batch, columns (b, t) b-major
        cs = slice(c * 256, (c + 1) * 256)
        xf0 = np.ascontiguousarray(x0[:, cs, :].reshape(B * 256, E).T)
        xf1 = np.ascontiguousarray(x1[:, cs, :].reshape(B * 256, E).T)
        m["x0t_ffn"] = xf0.astype(bf16)
        m["x1t_ffn"] = xf1.astype(bf16)
        m["x0t_ffn32"] = xf0
        m["x1t_ffn32"] = xf1
        m["wqk"] = np.ascontiguousarray(Wqk[:, hs]).astype(bf16)
        m["bqk"] = bqk[hs].reshape(128, 1)
        wvp = np.zeros((E, 256), np.float32)
        wvp[:, 0:64] = Wv[:, hg * 128:hg * 128 + 64]
        wvp[:, 192:256] = Wv[:, hg * 128 + 64:hg * 128 + 128]
        m["wv"] = wvp.astype(bf16)
        in_maps.append(m)
    return in_maps


def _get_nc():
    if "nc" not in _cache:
        _cache["nc"] = _build()
    return _cache["nc"]


def kernel(**inputs):
    from concourse import bass_utils
    nc = _get_nc()
    in_maps = _host_prep(inputs)
    res = bass_utils.run_bass_kernel_spmd(nc, in_maps, core_ids=list(range(N_CORES)))
    out0 = np.empty((B, NT, E), np.float32)
    out1 = np.empty((B, NT, E), np.float32)
    for c in range(N_CORES):
        cs = slice(c * 256, (c + 1) * 256)
        o0 = res.results[c]["out0t"]  # [E, 4*256], cols (b, t)
        o1 = res.results[c]["out1t"]
        for b in range(B):
            out0[b, cs, :] = o0[:, b * 256:(b + 1) * 256].T
            out1[b, cs, :] = o1[:, b * 256:(b + 1) * 256].T
    return out0, out1


# revision 17
# speedup vs baseline: 1.3843x; 1.0195x over previous
"""CrossTransformer Trainium2 kernel — 8 NeuronCores (bf16 compute).

Sharding: core c = (batch b = c//2, head-pair hg = c%2).  Attention is
head-parallel (2 heads/core, bf16 matmuls, exp on ACT with fused
row-sum accum); out-proj + FFN are token-parallel (a 256-token slice of
every batch per core) after 8-way AllToAlls of the attention output.

The exchange is split per path: path-0's AllToAll (raw m + exp row
sums) is issued mid-kernel and runs during path-1's attention, hiding
the collective's entry-barrier skew.  Softmax normalization happens
post-exchange (scale commutes past Wo), via DVE reciprocal + a
ones-matmul partition broadcast.

Bias folding (host side): bv folds through Wo into bo
(bo_eff = bo + bv@Wo), and bo_eff folds through W1 into b1
(b1_eff = b1 + bo_eff@W1[E:]), so the v-proj and out-proj carry no
bias at all on device.
"""
import numpy as np

B, NT, E, H, D = 4, 2048, 256, 4, 64
HPC = 2            # heads per core
TH = NT // 2       # token half (per-core FFN token count = 4*256)
HID = 2 * E        # FFN hidden (512)
KCH = E // 128     # 128-chunks of E (2)
N_CORES = 8
LN_EPS = 1e-5

_cache = {}


def _build():
    import concourse.bass as bass
    import concourse.tile as tile
    from concourse import bacc
    import concourse.mybir as mybir

    dt = mybir.dt
    f32, bf16 = dt.float32, dt.bfloat16

    nc = bacc.Bacc("TRN2", target_bir_lowering=False, debug=False,
                   num_devices=N_CORES)

    def din(name, shape, dtype):
        return nc.dram_tensor(name, shape, dtype, kind="ExternalInput").ap()

    t = dict(
        x0t=din("x0t", [E, NT], bf16),          # x0[b].T
        x1t=din("x1t", [E, NT], bf16),
        xfb=[din(f"x{d}t_ffn", [E, TH], bf16) for d in (0, 1)],
        xff=[din(f"x{d}t_ffn32", [E, TH], f32) for d in (0, 1)],
        wqk=din("wqk", [E, 128], bf16),         # pre-scaled, this core's heads
        bqk=din("bqk", [128, 1], f32),
        wv=din("wv", [E, 256], bf16),           # head0->cols 0:64, head1->192:256
        wo=din("wo", [E, E], bf16),
        w1=din("w1", [HID, HID], bf16),
        w1bar=din("w1bar", [HID, 1], bf16),
        b1=din("b1", [HID, 1], f32),            # b1_eff
        b1bar=din("b1bar", [1, 1], f32),        # b1bar_eff
        lng=din("lng", [HID, 1], f32),
        lnb=din("lnb", [HID, 1], f32),
        w2=din("w2", [HID, E], bf16),
        b2=din("b2", [E, 1], f32),
        ident=din("ident", [128, 128], bf16),
        ones=din("ones", [128, 1], bf16),       # column of ones (ss rowsums)
        ones1=din("ones1", [1, 128], bf16),     # single-partition row of ones
        ones1r=din("ones1r", [1, 128], dt.float32r),
        outs=[nc.dram_tensor(f"out{d}t", [E, TH], f32,
                             kind="ExternalOutput").ap() for d in (0, 1)],
    )

    with tile.TileContext(nc) as tc:
        _body(nc, tc, bass, mybir, tile, t)
    nc.compile()
    return nc


def _body(nc, tc, bass, mybir, tile, t):
    from contextlib import ExitStack
    dt = mybir.dt
    AF = mybir.ActivationFunctionType
    OP = mybir.AluOpType
    f32, bf16 = dt.float32, dt.bfloat16

    es = ExitStack()
    with es:
        wpool = es.enter_context(tc.tile_pool(name="weights", bufs=1))
        dram = es.enter_context(tc.tile_pool(name="dram", bufs=1, space="DRAM"))

        def load(ap_src, p, fshape, tag, dtype, eng):
            til = wpool.tile([p, fshape], dtype, tag=tag, name=tag)
            eng.dma_start(til[:], ap_src)
            return til

        S, G = nc.sync, nc.gpsimd
        # long-lived pools open before the projection-scoped xtp (LIFO)
        rows = es.enter_context(tc.tile_pool(name="rows", bufs=1))
        attn_es = ExitStack()
        qp = attn_es.enter_context(tc.tile_pool(name="qkv", bufs=1))
        proj_es = ExitStack()
        xtp = proj_es.enter_context(tc.tile_pool(name="xtp", bufs=1))
        # critical-path loads first: ident (warm-up), wqk + x (qk proj)
        ident_t = load(t["ident"], 128, 128, "ident", bf16, S)
        wqk_t = [load(t["wqk"][k * 128:(k + 1) * 128, :], 128, 128, f"wqk{k}", bf16, S)
                 for k in range(KCH)]
        xt = [[None, None], [None, None]]
        for s, src in enumerate((t["x0t"], t["x1t"])):
            for k in range(KCH):
                xt[s][k] = xtp.tile([128, NT], bf16, tag=f"x{s}{k}", name=f"x{s}{k}")
                (S if s == 0 else G).dma_start(xt[s][k][:], src[k * 128:(k + 1) * 128, :])
        wv_t = [load(t["wv"][k * 128:(k + 1) * 128, :], 128, 256, f"wv{k}", bf16, S)
                for k in range(KCH)]
        bqk_t = load(t["bqk"], 128, 1, "bqk", f32, S)
        ones_t = load(t["ones"], 128, 1, "ones", bf16, G)
        ones1_t = load(t["ones1"], 1, 128, "ones1", bf16, G)
        ones1r_t = load(t["ones1r"], 1, 128, "ones1r", dt.float32r, G)
        wo_t = [[load(t["wo"][k * 128:(k + 1) * 128, m * 128:(m + 1) * 128], 128, 128,
                      f"wo{k}{m}", bf16, G) for m in range(2)] for k in range(KCH)]
        w1_t = [[load(t["w1"][k * 128:(k + 1) * 128, m * 128:(m + 1) * 128], 128, 128,
                      f"w1{k}{m}", bf16, G) for m in range(4)] for k in range(4)]
        w2_t = [[load(t["w2"][k * 128:(k + 1) * 128, m * 128:(m + 1) * 128], 128, 128,
                      f"w2{k}{m}", bf16, G) for m in range(2)] for k in range(4)]
        w1bar_t = [load(t["w1bar"][k * 128:(k + 1) * 128, :], 128, 1, f"w1b{k}", bf16, G)
                   for k in range(4)]
        b1_t = [load(t["b1"][m * 128:(m + 1) * 128, :], 128, 1, f"b1_{m}", f32, G)
                for m in range(4)]
        b1bar_t = load(t["b1bar"], 1, 1, "b1bar", f32, G)
        lng_t = [load(t["lng"][m * 128:(m + 1) * 128, :], 128, 1, f"lng{m}", f32, G)
                 for m in range(4)]
        lnb_t = [load(t["lnb"][m * 128:(m + 1) * 128, :], 128, 1, f"lnb{m}", f32, G)
                 for m in range(4)]
        b2_t = [load(t["b2"][m * 128:(m + 1) * 128, :], 128, 1, f"b2_{m}", f32, G)
                for m in range(2)]
        xfb = [[load(t["xfb"][d][k * 128:(k + 1) * 128, :], 128, TH, f"xfb{d}{k}", bf16, G)
                for k in range(KCH)] for d in range(2)]

        # ---- PE warm-up burst: drive HAM to K=8/8 while DMAs stream ----
        with tc.tile_pool(name="warmps", bufs=1, space="PSUM") as wps:
            warm = wps.tile([128, 128], f32)
            for _ in range(64):
                nc.tensor.matmul(warm[:], ident_t[:], ident_t[:],
                                 start=True, stop=True)

        # ================= projections =================
        qkT = [None, None]   # [128(2h*64d), NT] bf16
        v_t = [[None] * 16, [None] * 16]   # 16 x [128 tok, 256(h0|0|0|h1)] bf16
        with tc.tile_pool(name="qkps", bufs=1, space="PSUM") as qkps, \
             tc.tile_pool(name="vps", bufs=3, space="PSUM") as vps:
            for s in range(2):
                ps = qkps.tile([128, NT], f32, tag="qkps")
                for jn in range(NT // 512):
                    for k in range(KCH):
                        nc.tensor.matmul(ps[:, jn * 512:(jn + 1) * 512],
                                         wqk_t[k][:], xt[s][k][:, jn * 512:(jn + 1) * 512],
                                         start=(k == 0), stop=(k == KCH - 1))
                qkT[s] = qp.tile([128, NT], bf16, tag=f"qkT{s}", name=f"qkT{s}")
                nc.vector.tensor_scalar(qkT[s][:], ps[:], bqk_t[:], None, OP.add)
            for s in range(2):
                for it in range(16):
                    pv = vps.tile([128, 256], f32, tag="vps")
                    for k in range(KCH):
                        nc.tensor.matmul(pv[:], xt[s][k][:, it * 128:(it + 1) * 128],
                                         wv_t[k][:], start=(k == 0), stop=(k == KCH - 1))
                    v_t[s][it] = qp.tile([128, 256], bf16, tag=f"v{s}_{it}", name=f"v{s}_{it}")
                    if s == 0:
                        nc.scalar.copy(v_t[s][it][:], pv[:])
                    else:
                        nc.vector.tensor_copy(v_t[s][it][:], pv[:])
        proj_es.close()   # frees the x-transpose tiles

        # bounce layout per path: [8 dest-core blocks, 130, 256] bf16
        # rows 0:128 = raw m, rows 128:130 = exp row sums (hl = head in pair)
        bounce_in = [dram.tile([8, 130, 256], bf16, name=f"bnc_in{p}") for p in range(2)]
        bounce_out = [dram.tile([8, 130, 256], bf16, name=f"bnc_out{p}") for p in range(2)]

        # ================= attention (two symmetric paths) =================
        # path p: (A,B) = (p, 1-p); est = exp(qkA^T qkB) [tokA, tokB];
        # m_raw[p][feat, tokB] = sum_tokA v_A est.
        # rs_p (row sums over tokB per tokA) ship with path p's m; the
        # denominator for m_raw[p] (indexed by tokB) is rs_{1-p}.
        mrp = attn_es.enter_context(tc.tile_pool(name="mraw", bufs=1))
        rsj = [rows.tile([128, 64], f32, tag=f"rsj{jj}", name=f"rsj{jj}")
               for jj in range(2)]
        m_raw = [None, None]
        with tc.tile_pool(name="estrip", bufs=4) as ep, \
             tc.tile_pool(name="simps", bufs=2, space="PSUM") as simps, \
             tc.tile_pool(name="avps", bufs=2, space="PSUM") as avps, \
             tc.tile_pool(name="trps", bufs=1, space="PSUM") as trps:
            for p in range(2):
                A, Bi = p, 1 - p
                m_raw[p] = mrp.tile([128, NT], bf16, tag=f"mraw{p}", name=f"mraw{p}")
                for jj in range(2):
                    av = [avps.tile([128, 512], f32, tag="av",
                                    name=f"av{p}_{jj}_{_i}") for _i in range(2)]
                    for it in range(16):
                        est = [None, None]
                        for h in range(2):
                            sp = simps.tile([128, 1024], f32, tag="sim")
                            for jc in range(2):
                                nc.tensor.matmul(
                                    sp[:, jc * 512:(jc + 1) * 512],
                                    qkT[A][64 * h:64 * (h + 1), it * 128:(it + 1) * 128],
                                    qkT[Bi][64 * h:64 * (h + 1),
                                            jj * 1024 + jc * 512:jj * 1024 + (jc + 1) * 512],
                                    start=True, stop=True,
                                    tile_position=(64 * h, 0))
                            est[h] = ep.tile([128, 1024], bf16, tag="est", name=f"est{h}")
                            # col layout (p, h, tck=it%2, j=it//2): makes the
                            # transposed row sums contiguous per A2A shard
                            col = p * 32 + h * 16 + (it % 2) * 8 + it // 2
                            nc.scalar.activation(est[h][:], sp[:], AF.Exp,
                                                 accum_out=rsj[jj][:, col:col + 1])
                        for h in range(2):
                            # lhsT = zero-padded v half h: product rows
                            # 64h:64h+64 get head h's AV, other 64 rows zeros
                            for jc in range(2):
                                nc.tensor.matmul(
                                    av[jc][:],
                                    v_t[A][it][:, h * 128:(h + 1) * 128],
                                    est[h][:, jc * 512:(jc + 1) * 512],
                                    start=(it == 0 and h == 0),
                                    stop=(it == 15 and h == 1))
                    for jc in range(2):
                        nc.vector.tensor_copy(
                            m_raw[p][:, jj * 1024 + jc * 512:jj * 1024 + (jc + 1) * 512],
                            av[jc][:])
                # path-p row sums -> [32(hl,it), 128] bf16, transposed on PE
                rsp = rows.tile([128, 32], f32, tag=f"rsp{p}", name=f"rsp{p}")
                nc.vector.tensor_add(rsp[:], rsj[0][:, p * 32:(p + 1) * 32],
                                     rsj[1][:, p * 32:(p + 1) * 32])
                rsp_bf = rows.tile([128, 32], bf16, tag=f"rspb{p}", name=f"rspb{p}")
                nc.vector.tensor_copy(rsp_bf[:], rsp[:])
                tp = trps.tile([32, 128], f32, tag="tp")
                nc.tensor.matmul(tp[:], rsp_bf[:], ident_t[:], start=True, stop=True)
                rsT = rows.tile([32, 128], bf16, tag=f"rsT{p}", name=f"rsT{p}")
                nc.vector.tensor_copy(rsT[:], tp[:])
                # bounce: m blocks + rs rows, then this path's AllToAll
                for j in range(8):
                    nc.sync.dma_start(bounce_in[p][j, 0:128, :],
                                      m_raw[p][:, j * 256:(j + 1) * 256])
                for hl in range(2):
                    for tck in range(2):
                        # rsT partition hl*16 + tck*8 + j -> shard j, row 128+hl
                        nc.sync.dma_start(
                            bounce_in[p][:, 128 + hl, tck * 128:(tck + 1) * 128],
                            rsT[hl * 16 + tck * 8:hl * 16 + (tck + 1) * 8, :])
                if p == 0:
                    # path-0 exchange runs during path-1's attention
                    nc.gpsimd.collective_compute(
                        "AllToAll", mybir.AluOpType.bypass,
                        replica_groups=[list(range(8))],
                        ins=[bounce_in[0].opt()], outs=[bounce_out[0].opt()])
        attn_es.close()   # frees qkT/v/m_raw SBUF for the FFN phase

        # ---- FFN x-part: no exchange dependency, fills the A2A wait ----
        hx = {}
        mux = {}
        hsb_pool = es.enter_context(tc.tile_pool(name="hsb", bufs=1))
        statp = es.enter_context(tc.tile_pool(name="statrows", bufs=1))
        with tc.tile_pool(name="hxps", bufs=2, space="PSUM") as hxps, \
             tc.tile_pool(name="rowxps", bufs=1, space="PSUM") as rowxps:
            for d in range(2):
                for tcn in range(2):
                    sl = slice(tcn * 512, (tcn + 1) * 512)
                    pmu = rowxps.tile([1, 512], f32, tag="pmux")
                    for kc in range(KCH):
                        nc.tensor.matmul(pmu[:], w1bar_t[kc][:], xfb[d][kc][:, sl],
                                         start=(kc == 0), stop=(kc == KCH - 1))
                    mux[(d, tcn)] = statp.tile([1, 512], f32, tag=f"mux{d}{tcn}",
                                               name=f"mux{d}{tcn}")
                    nc.vector.tensor_scalar(mux[(d, tcn)][:], pmu[:], b1bar_t[:],
                                            None, OP.add)
                    for mh in range(4):
                        ph = hxps.tile([128, 512], f32, tag="phx")
                        for kc in range(KCH):
                            nc.tensor.matmul(ph[:], w1_t[kc][mh][:], xfb[d][kc][:, sl],
                                             start=(kc == 0), stop=(kc == KCH - 1))
                        key = (d, tcn, mh)
                        hx[key] = hsb_pool.tile([128, 512], bf16, tag=f"hx{d}{tcn}{mh}",
                                                name=f"hx{d}{tcn}{mh}")
                        nc.vector.tensor_scalar(hx[key][:], ph[:], b1_t[mh][:],
                                                None, OP.add)

        # path-1 exchange, emitted after the x-part so the x-part matmuls
        # fill this collective's entry-barrier wait
        nc.gpsimd.collective_compute(
            "AllToAll", mybir.AluOpType.bypass,
            replica_groups=[list(range(8))],
            ins=[bounce_in[1].opt()], outs=[bounce_out[1].opt()])

        # ---- gather exchanged m + rs; normalize mdir; out-projection ----
        mpool = es.enter_context(tc.tile_pool(name="mdir", bufs=1))
        mdir = [[None, None], [None, None]]
        for d in range(2):
            p = 1 - d
            for kc in range(2):
                mdir[d][kc] = mpool.tile([128, TH], bf16, tag=f"mdir{d}{kc}",
                                         name=f"mdir{d}{kc}")
                for bb in range(4):
                    nc.sync.dma_start(
                        mdir[d][kc][:, bb * 256:(bb + 1) * 256],
                        bounce_out[p][bb * 2 + kc, 0:128, :])
        # rs for direction d's tokens = path d's rs (rows of est_d).
        # reciprocal as [8 (src j), 512 (hl*256+t)], then flattened onto a
        # single partition via DRAM so matmul rhs slices start at partition 0
        rs_flat = [None, None]
        for p in range(2):
            rs_sb = mpool.tile([8, 512], bf16, tag=f"rssb{p}", name=f"rssb{p}")
            nc.sync.dma_start(
                rs_sb[:],
                bounce_out[p][:, 128:130, :].rearrange("j hl t -> j (hl t)"))
            rsf = mpool.tile([8, 512], f32, tag=f"rsf{p}", name=f"rsf{p}")
            nc.vector.reciprocal(rsf[:], rs_sb[:])
            rs_rec = mpool.tile([8, 512], bf16, tag=f"rsrec{p}", name=f"rsrec{p}")
            nc.vector.tensor_copy(rs_rec[:], rsf[:])
            scratch = dram.tile([8, 512], bf16, name=f"rs_scr{p}")
            nc.sync.dma_start(scratch[:], rs_rec[:])
            rs_flat[p] = mpool.tile([1, 4096], bf16, tag=f"rsfl{p}", name=f"rsfl{p}")
            nc.sync.dma_start(rs_flat[p][:], scratch.rearrange("j c -> (j c)"))
        with tc.tile_pool(name="rbps", bufs=1, space="PSUM") as rbps:
            rb = {}
            for d in range(2):
                for kc in range(2):
                    rb[(d, kc)] = rbps.tile([128, TH], f32, tag=f"rb{d}{kc}",
                                            name=f"rb{d}{kc}")
                    for hl in range(2):
                        for bb in range(4):
                            nc.tensor.matmul(
                                rb[(d, kc)][64 * hl:64 * (hl + 1),
                                            bb * 256:(bb + 1) * 256],
                                ones1_t[0:1, 0:64],
                                rs_flat[d][0:1,
                                           (bb * 2 + kc) * 512 + hl * 256:
                                           (bb * 2 + kc) * 512 + (hl + 1) * 256],
                                start=True, stop=True,
                                tile_position=(0, 64 * hl))
            for d in range(2):
                for kc in range(2):
                    nc.vector.tensor_mul(mdir[d][kc][:], mdir[d][kc][:],
                                         rb[(d, kc)][:])
        mproj = [[None, None], [None, None]]
        with tc.tile_pool(name="mpps", bufs=2, space="PSUM") as mpps:
            for d in range(2):
                for mo in range(2):
                    po = mpps.tile([128, TH], f32, tag="mp")
                    for nn in range(2):
                        for kc in range(2):
                            nc.tensor.matmul(po[:, nn * 512:(nn + 1) * 512],
                                             wo_t[kc][mo][:],
                                             mdir[d][kc][:, nn * 512:(nn + 1) * 512],
                                             start=(kc == 0), stop=(kc == 1))
                    mproj[d][mo] = mpool.tile([128, TH], bf16, tag=f"mproj{d}{mo}",
                                              name=f"mproj{d}{mo}")
                    nc.vector.tensor_copy(mproj[d][mo][:], po[:])

        # ================= FFN m-part + LN stats (batched rows) ==========
        hsb = {}
        t2 = {}
        gh = {}
        mu_all = statp.tile([1, 2048], f32, tag="muall", name="mu_all")
        var_all = statp.tile([1, 2048], f32, tag="varall", name="var_all")
        with tc.tile_pool(name="sq", bufs=3) as sqp, \
             tc.tile_pool(name="t2p", bufs=4) as t2p, \
             tc.tile_pool(name="hps", bufs=2, space="PSUM") as hps, \
             tc.tile_pool(name="rowps", bufs=1, space="PSUM") as rowps, \
             tc.tile_pool(name="bps", bufs=1, space="PSUM") as bps, \
             tc.tile_pool(name="gacts", bufs=8) as gp, \
             tc.tile_pool(name="osb", bufs=4) as op_, \
             tc.tile_pool(name="ops", bufs=2, space="PSUM") as ops:
            for d in range(2):
                for tcn in range(2):
                    sl = slice(tcn * 512, (tcn + 1) * 512)
                    col = (d * 2 + tcn) * 512
                    pmu = rowps.tile([1, 512], f32, tag="pmu")
                    for kc in range(2):
                        nc.tensor.matmul(pmu[:], w1bar_t[2 + kc][:],
                                         mproj[d][kc][:, sl],
                                         start=(kc == 0), stop=(kc == 1))
                    nc.vector.tensor_add(mu_all[0:1, col:col + 512], pmu[:],
                                         mux[(d, tcn)][:])
                    pss = rowps.tile([1, 512], f32, tag="pss")
                    for mh in range(4):
                        ph = hps.tile([128, 512], f32, tag="ph")
                        for kc in range(2):
                            nc.tensor.matmul(ph[:], w1_t[2 + kc][mh][:],
                                             mproj[d][kc][:, sl],
                                             start=(kc == 0), stop=(kc == 1))
                        key = (d, tcn, mh)
                        hsb[key] = hsb_pool.tile([128, 512], bf16, tag=f"h{d}{tcn}{mh}",
                                                 name=f"h{d}{tcn}{mh}")
                        nc.vector.tensor_add(hsb[key][:], ph[:], hx[key][:])
                        sq = sqp.tile([128, 512], bf16, tag="sq")
                        nc.scalar.square(sq[:], hsb[key][:])
                        nc.tensor.matmul(pss[:], ones_t[:], sq[:],
                                         start=(mh == 0), stop=(mh == 3))
                    # var = ss/HID - mu^2 + eps
                    musq = statp.tile([1, 512], f32, tag="musq")
                    nc.vector.tensor_mul(musq[:], mu_all[0:1, col:col + 512],
                                         mu_all[0:1, col:col + 512])
                    nc.vector.scalar_tensor_tensor(var_all[0:1, col:col + 512],
                                                   pss[:], 1.0 / HID, musq[:],
                                                   OP.mult, OP.subtract)
            nc.vector.tensor_scalar(var_all[:], var_all[:], LN_EPS, None, OP.add)
            # rstd = exp(-0.5 ln(var)) — batched over all 4 blocks, one
            # table set (natural_log_exp) shared with the attention exps
            lnv = statp.tile([1, 2048], f32, tag="lnv", name="lnv")
            nc.scalar.activation(lnv[:], var_all[:], AF.Ln)
            rstd = statp.tile([1, 2048], dt.float32r, tag="rstd", name="rstd")
            nc.scalar.activation(rstd[:], lnv[:], AF.Exp, scale=-0.5)
            murstd = statp.tile([1, 2048], dt.float32r, tag="murstd", name="murstd")
            nc.vector.tensor_mul(murstd[:], mu_all[:], rstd[:])
            for d in range(2):
                for tcn in range(2):
                    col = (d * 2 + tcn) * 512
                    rstdb = bps.tile([128, 512], f32, tag="rstdb")
                    nc.tensor.matmul(rstdb[:], ones1r_t[:],
                                     rstd[0:1, col:col + 512],
                                     start=True, stop=True)
                    murstdb = bps.tile([128, 512], f32, tag="murstdb")
                    nc.tensor.matmul(murstdb[:], ones1r_t[:],
                                     murstd[0:1, col:col + 512],
                                     start=True, stop=True)
                    for mh in range(4):
                        key = (d, tcn, mh)
                        u = sqp.tile([128, 512], bf16, tag="sq")
                        nc.vector.tensor_mul(u[:], hsb[key][:], rstdb[:])
                        t2[key] = t2p.tile([128, 512], bf16, tag="t2", name="t2")
                        nc.vector.tensor_sub(t2[key][:], u[:], murstdb[:])
                        # gelus stay contiguous on ACT (affines are DVE-only)
                        gh[key] = gp.tile([128, 512], bf16, tag="gh", name="gh")
                        nc.scalar.activation(gh[key][:], t2[key][:], AF.Gelu,
                                             bias=lnb_t[mh][:], scale=lng_t[mh][:])
            xffs = {}
            for d in range(2):
                for mo in range(2):
                    xffs[(d, mo)] = op_.tile([128, TH], f32, tag="xffs",
                                             bufs=2, name="xffs")
                    nc.sync.dma_start(
                        xffs[(d, mo)][:],
                        t["xff"][d][mo * 128:(mo + 1) * 128, :])
            for d in range(2):
                for tcn in range(2):
                    sl = slice(tcn * 512, (tcn + 1) * 512)
                    for mo in range(2):
                        po = ops.tile([128, 512], f32, tag="po")
                        for kh in range(4):
                            nc.tensor.matmul(po[:], w2_t[kh][mo][:],
                                             gh[(d, tcn, kh)][:],
                                             start=(kh == 0), stop=(kh == 3))
                        ot = op_.tile([128, 512], f32, tag="ot")
                        nc.vector.scalar_tensor_tensor(
                            ot[:], po[:], b2_t[mo][:], xffs[(d, mo)][:, sl],
                            OP.add, OP.add)
                        nc.sync.dma_start(t["outs"][d][mo * 128:(mo + 1) * 128, sl],
                                          ot[:])


def _host_prep(inputs):
    """Build per-core in_maps from full inputs (bf16 matmul operands)."""
    import ml_dtypes
    bf16 = ml_dtypes.bfloat16
    x0 = np.asarray(inputs["x0"], np.float32)
    x1 = np.asarray(inputs["x1"], np.float32)
    Wqk = np.asarray(inputs["Wqk"], np.float32) * (D ** -0.25)
    bqk = np.asarray(inputs["bqk"], np.float32) * (D ** -0.25)
    Wv = np.asarray(inputs["Wv"], np.float32)
    bv = np.asarray(inputs["bv"], np.float32)
    Wo = np.asarray(inputs["Wo"], np.float32)
    bo = np.asarray(inputs["bo"], np.float32)
    W1 = np.asarray(inputs["W1"], np.float32)
    b1 = np.asarray(inputs["b1"], np.float32)
    lng = np.asarray(inputs["ln_g"], np.float32)
    lnb = np.asarray(inputs["ln_b"], np.float32)
    W2 = np.asarray(inputs["W2"], np.float32)
    b2 = np.asarray(inputs["b2"], np.float32)

    # fold bv through Wo, then bo_eff through W1's m-half
    bo_eff = bo + bv @ Wo
    b1_eff = b1 + bo_eff @ W1[E:, :]

    shared = {
        "wo": Wo.astype(bf16),
        "w1": W1.astype(bf16),
        "w1bar": W1.mean(axis=1).reshape(HID, 1).astype(bf16),
        "b1": b1_eff.reshape(HID, 1),
        "b1bar": np.array([[b1_eff.mean()]], np.float32),
        "lng": lng.reshape(HID, 1),
        "lnb": lnb.reshape(HID, 1),
        "w2": W2.astype(bf16),
        "b2": b2.reshape(E, 1),
        "ident": np.eye(128, dtype=bf16),
        "ones": np.ones((128, 1), bf16),
        "ones1": np.ones((1, 128), bf16),
        "ones1r": np.ones((1, 128), np.float32),
    }
    in_maps = []
    for c in range(N_CORES):
        b, hg = c // 2, c % 2
        hs = slice(hg * 128, hg * 128 + 128)
        m = dict(shared)
        m["x0t"] = np.ascontiguousarray(x0[b].T).astype(bf16)
        m["x1t"] = np.ascontiguousarray(x1[b].T).astype(bf16)
        # FFN slice: my 256-token slice of EVERY batch, columns (b, t) b-major
        cs = slice(c * 256, (c + 1) * 256)
        xf0 = np.ascontiguousarray(x0[:, cs, :].reshape(B * 256, E).T)
        xf1 = np.ascontiguousarray(x1[:, cs, :].reshape(B * 256, E).T)
        m["x0t_ffn"] = xf0.astype(bf16)
        m["x1t_ffn"] = xf1.astype(bf16)
        m["x0t_ffn32"] = xf0
        m["x1t_ffn32"] = xf1
        m["wqk"] = np.ascontiguousarray(Wqk[:, hs]).astype(bf16)
        m["bqk"] = bqk[hs].reshape(128, 1)
        wvp = np.zeros((E, 256), np.float32)
        wvp[:, 0:64] = Wv[:, hg * 128:hg * 128 + 64]
        wvp[:, 192:256] = Wv[:, hg * 128 + 64:hg * 128 + 128]
        m["wv"] = wvp.astype(bf16)
        in_maps.append(m)
    return in_maps


def _get_nc():
    if "nc" not in _cache:
        _cache["nc"] = _build()
    return _cache["nc"]


def kernel(**inputs):
    from concourse import bass_utils
    nc = _get_nc()
    in_maps = _host_prep(inputs)
    res = bass_utils.run_bass_kernel_spmd(nc, in_maps, core_ids=list(range(N_CORES)))
    out0 = np.empty((B, NT, E), np.float32)
    out1 = np.empty((B, NT, E), np.float32)
    for c in range(N_CORES):
        cs = slice(c * 256, (c + 1) * 256)
        o0 = res.results[c]["out0t"]  # [E, 4*256], cols (b, t)
        o1 = res.results[c]["out1t"]
        for b in range(B):
            out0[b, cs, :] = o0[:, b * 256:(b + 1) * 256].T
            out1[b, cs, :] = o1[:, b * 256:(b + 1) * 256].T
    return out0, out1
